# revision 3
# baseline (speedup 1.0000x reference)
"""Multi-head attention (B=8, N=1024, C=768, H=12, D=64) on 8 TRN2
NeuronCores, data-parallel over batch. v2 restructure of the baseline.

Per-core dataflow (all matmuls bf16, PSUM f32):
  x[1024,768] --DMA--> stage f32 --DVE cast--> bf16 --PE transpose--> xT
  qkv: pq[t][128,1024] = W-pair-tile^T x  (dims of heads 2t,2t+1 stacked)
       q2[t] = pq + bq (DVE scalar-col add), k2[t] plain copy. Softmax
       over keys m is invariant to per-query constants, so k-bias (whose
       logit term q.bk is constant per query) is dropped entirely; the
       q-bias supplies the needed bq.k_m term.
  v_aug[mt][128, h*128 + (64 v | 64 ones)] (v-bias folded into proj bias)
  per head h (t=h//2, r0=64*(h%2)):
    scoresT[m,n]: K=64 matmuls k2[t][r0:r0+64, msl]^T @ q2[t][r0:r0+64, nsl]
    expT = Exp(SCALE*scoresT) on ACT, [128,1024] chunks
    pav = v_aug^T @ expT  (rows 0:63 attn@v, 64:127 softmax denominator)
    bc = exp(-ln(den)) on ACT; aoT[t][r0:r0+64] = pav[0:64]*bc (DVE)
  proj: po = aoT^T pwT; out = po + pbias_bcast (DVE add) -> DMA
Host folds: pb_full = proj_b + proj_w @ bv."""
import sys

sys.path.insert(0, "/opt/trn_rl_repo")

from contextlib import ExitStack

import ml_dtypes
import numpy as np

import concourse.bass as bass
import concourse.mybir as mybir
import concourse.tile as tile
from concourse.bass_utils import run_bass_kernel_spmd
from concourse.masks import make_identity

DIM = 768
HEADS = 12
HD = 64
N = 1024
SCALE = HD ** -0.5
P = 128
NT = N // P          # 8 n-tiles
CT = DIM // P        # 6 c-tiles
PAIRS = HEADS // 2   # 6 head-pairs
F32 = mybir.dt.float32
F32R = mybir.dt.float32r
BF16 = mybir.dt.bfloat16
Exp = mybir.ActivationFunctionType.Exp
Ln = mybir.ActivationFunctionType.Ln
Mult = mybir.AluOpType.mult
Add = mybir.AluOpType.add

N_CORES = 8
WCOLS = 3 * DIM


def build_nc():
    nc = bass.Bass(trn_type="TRN2", target_bir_lowering=False, debug=False,
                   enable_asserts=False)
    x_d = nc.declare_dram_parameter("x", [N, DIM], F32, isOutput=False).ap()
    qkvwt_d = nc.declare_dram_parameter("qkv_wT", [DIM, WCOLS], BF16, isOutput=False).ap()
    qkvb_d = nc.declare_dram_parameter("qkv_b", [DIM], F32, isOutput=False).ap()
    projwt_d = nc.declare_dram_parameter("proj_wT", [DIM, DIM], BF16, isOutput=False).ap()
    projb_d = nc.declare_dram_parameter("proj_b", [DIM], F32, isOutput=False).ap()
    out_d = nc.declare_dram_parameter("out", [N, DIM], F32, isOutput=True).ap()

    with tile.TileContext(nc) as tc, ExitStack() as top:
        const = top.enter_context(tc.tile_pool(name="const", bufs=1))
        identity = const.tile([P, P], F32)
        make_identity(nc, identity[:])
        identity_b = const.tile([P, P], BF16)
        nc.vector.tensor_copy(identity_b[:], identity[:])
        pbias = const.tile([P, DIM], F32)  # proj bias broadcast to 128 rows

        # persistent activations / weights
        xw = top.enter_context(tc.tile_pool(name="xw", bufs=1))
        xT = xw.tile([P, CT * N], BF16, tag="xT", name="xT")
        wT = [xw.tile([P, WCOLS], BF16, tag=f"wT{i}", name=f"wT{i}") for i in range(CT)]
        pwT = [xw.tile([P, DIM], BF16, tag=f"pwT{i}", name=f"pwT{i}") for i in range(CT)]
        q2 = [xw.tile([P, N], BF16, tag=f"q2{t}", name=f"q2{t}") for t in range(PAIRS)]
        k2 = [xw.tile([P, N], BF16, tag=f"k2{t}", name=f"k2{t}") for t in range(PAIRS)]
        v_aug = [xw.tile([P, HEADS * P], BF16, tag=f"v{i}", name=f"v{i}") for i in range(NT)]
        bqcol = xw.tile([P, PAIRS], F32, tag="bqcol", name="bqcol")
        aoT = [xw.tile([P, N], BF16, tag=f"aoT{i}", name=f"aoT{i}") for i in range(CT)]

        for nt in range(NT):
            va3 = v_aug[nt][:].rearrange("p (h e) -> p h e", e=P)
            nc.gpsimd.memset(va3[:, :, HD:P], 1.0)

        def xts(ct, n0, n1):
            return xT[:, ct * N + n0: ct * N + n1]

        with tc.tile_pool(name="prtp", bufs=NT) as prtp, \
             tc.tile_pool(name="expp", bufs=3) as expp, \
             tc.tile_pool(name="bcp", bufs=2) as bcp, \
             tc.tile_pool(name="outp", bufs=2) as outp, \
             tc.tile_pool(name="psum_big", bufs=2, space="PSUM") as psum_big:
            stage_cm = tc.tile_pool(name="stage", bufs=3)
            psum_t_cm = tc.tile_pool(name="psum_t", bufs=2, space="PSUM")
            stage = stage_cm.__enter__()
            psum_t = psum_t_cm.__enter__()
            # pbias = broadcast of proj_b_full over 128 rows (ones-col matmul)
            brow_f = const.tile([1, DIM], F32, tag="brow", name="brow")
            nc.sync.dma_start(brow_f[:], projb_d.unsqueeze(0))
            nc.sync.dma_start(bqcol[:], qkvb_d.rearrange("(o p) -> p o", p=P))
            ones_f = const.tile([1, P], F32, tag="ones_f", name="ones_f")
            nc.vector.memset(ones_f[:], 1.0)
            ones_r = const.tile([1, P], F32R, tag="ones_r", name="ones_r")
            nc.vector.tensor_copy(ones_r[:], ones_f[:])
            brow_r = const.tile([1, DIM], F32R, tag="brow_r", name="brow_r")
            nc.vector.tensor_copy(brow_r[:], brow_f[:])
            pb_big = psum_big.tile([P, N], F32, tag="pqk", name="pbps")
            pb_ps = pb_big[:, 0:DIM]
            for o0, osz in ((0, 512), (512, 256)):
                nc.tensor.matmul(pb_big[:, o0:o0 + osz], ones_r[0:1, :],
                                 brow_r[0:1, o0:o0 + osz], start=True, stop=True)
            nc.vector.tensor_copy(pbias[:], pb_big[:, 0:DIM])

            sts, stbs, pts = [None] * NT, [None] * NT, [None] * NT

            def xdma(rt):
                sts[rt] = stage.tile([P, DIM], F32, tag="st", name=f"st{rt}")
                nc.sync.dma_start(sts[rt][:], x_d[rt * P:(rt + 1) * P, :])

            nc.sync.dma_start(wT[0][:], qkvwt_d[0:P, :])
            for rt in range(NT):
                xdma(rt)
                if rt == 0:
                    for ct in range(1, CT):
                        nc.sync.dma_start(wT[ct][:], qkvwt_d[ct * P:(ct + 1) * P, :])
            for ct in range(CT):
                nc.sync.dma_start(pwT[ct][:], projwt_d[ct * P:(ct + 1) * P, :])

            def cast(rt):
                stbs[rt] = stage.tile([P, DIM], BF16, tag="stb", name=f"stb{rt}")
                nc.vector.tensor_copy(stbs[rt][:], sts[rt][:])

            def trans(rt):
                pts[rt] = psum_t.tile([P, DIM], BF16, tag="pt", name=f"pt{rt}")
                for ct in range(CT):
                    nc.tensor.transpose(pts[rt][:, ct * P:(ct + 1) * P],
                                        stbs[rt][:, ct * P:(ct + 1) * P],
                                        identity_b[:])

            def xtcopy(rt):
                nc.vector.tensor_copy(
                    xT[:].rearrange("p (c n) -> p c n", c=CT)[:, :, rt * P:(rt + 1) * P],
                    pts[rt][:].rearrange("p (c n) -> p c n", c=CT))

            # ---- attention emit helpers ----
            pq_live = {}

            def qk_mm(t, which, nch):
                # accumulate one 512-col chunk of pair t's q (which=0) or k dims
                key = (t, which)
                if key not in pq_live:
                    pq_live[key] = psum_big.tile([P, N], F32, tag="pqk",
                                                 name=f"pq{t}_{which}")
                pq = pq_live[key]
                wcol0 = which * DIM + t * P
                sl = slice(nch * 512, (nch + 1) * 512)
                for ct in range(CT):
                    nc.tensor.matmul(
                        pq[:, sl],
                        wT[ct][:, wcol0:wcol0 + P],
                        xts(ct, nch * 512, (nch + 1) * 512),
                        start=(ct == 0), stop=(ct == CT - 1))

            def qk_copy(t, which, nch):
                pq = pq_live[(t, which)]
                sl = slice(nch * 512, (nch + 1) * 512)
                dst = (q2 if which == 0 else k2)[t]
                if which == 0:
                    nc.vector.tensor_scalar_add(dst[:, sl], pq[:, sl],
                                                bqcol[:, t:t + 1])
                else:
                    nc.vector.tensor_copy(dst[:, sl], pq[:, sl])

            def qk_done(t, which):
                pq_live.pop((t, which))

            def emit_qk_half(t, which):
                qk_mm(t, which, 0)
                qk_mm(t, which, 1)
                qk_copy(t, which, 0)
                qk_copy(t, which, 1)
                qk_done(t, which)

            def emit_v(nt):
                pv = psum_big.tile([P, DIM], F32, tag="pqk", name=f"pv{nt}")
                for o0, osz in ((0, 512), (512, 256)):
                    for ct in range(CT):
                        nc.tensor.matmul(
                            pv[:, o0:o0 + osz],
                            xts(ct, nt * P, (nt + 1) * P),
                            wT[ct][:, 2 * DIM + o0:2 * DIM + o0 + osz],
                            start=(ct == 0), stop=(ct == CT - 1))
                va3 = v_aug[nt][:].rearrange("p (h e) -> p h e", e=P)
                nc.vector.tensor_copy(
                    va3[:, :, 0:HD],
                    pv[:].rearrange("p (h e) -> p h e", e=HD))

            scp = [None]

            def emit_scores_exp(h):
                t, r0 = h // 2, (h % 2) * HD
                expT = [expp.tile([P, N], BF16, tag=f"expT{mt}", name=f"expT{mt}_{h}")
                        for mt in range(NT)]
                for mt in range(NT):
                    ps = scp[0].tile([P, N], F32, tag="ps", name=f"ps{mt}_{h}")
                    for nch in range(2):
                        sl = slice(nch * 512, (nch + 1) * 512)
                        nc.tensor.matmul(
                            ps[:, sl],
                            k2[t][r0:r0 + HD, mt * P:(mt + 1) * P],
                            q2[t][r0:r0 + HD, sl],
                            start=True, stop=True)
                    nc.scalar.activation(expT[mt][:], ps[:], Exp, scale=SCALE)
                return expT

            def emit_av(h, expT):
                pav = psum_big.tile([P, N], F32, tag="pqk", name=f"pav{h}")
                for nch in range(2):
                    sl = slice(nch * 512, (nch + 1) * 512)
                    for mt in range(NT):
                        nc.tensor.matmul(
                            pav[:, sl],
                            v_aug[mt][:, h * P:(h + 1) * P],
                            expT[mt][:, sl],
                            start=(mt == 0), stop=(mt == NT - 1))
                return pav

            def emit_norm(h, pav):
                t, r0 = h // 2, (h % 2) * HD
                bc = bcp.tile([HD, N], F32, tag="bc", name=f"bc{h}")
                nc.scalar.activation(bc[:], pav[HD:P, :], Ln)
                nc.scalar.activation(bc[:], bc[:], Exp, scale=-1.0)
                nc.vector.tensor_tensor(aoT[t][r0:r0 + HD, :], pav[0:HD, :],
                                        bc[:], op=Mult)

            # ---- warmup: pipelined x prep interleaved with pair-0 qk ----
            cast(0)
            for rt in range(4):
                trans(rt)
                cast(rt + 1)
                xtcopy(rt)
            qk_mm(0, 0, 0)
            qk_mm(0, 1, 0)
            qk_copy(0, 0, 0)
            qk_copy(0, 1, 0)
            for rt in range(4, NT):
                trans(rt)
                if rt + 1 < NT:
                    cast(rt + 1)
                xtcopy(rt)
            qk_mm(0, 0, 1)
            qk_copy(0, 0, 1)
            qk_mm(0, 1, 1)
            qk_copy(0, 1, 1)
            qk_done(0, 0)
            qk_done(0, 1)
            psum_t_cm.__exit__(None, None, None)
            stage_cm.__exit__(None, None, None)
            psum_sc_cm = tc.tile_pool(name="psum_sc", bufs=2, space="PSUM")
            scp[0] = psum_sc_cm.__enter__()

            exps = {0: emit_scores_exp(0), 1: emit_scores_exp(1)}
            emit_qk_half(1, 0)
            emit_qk_half(1, 1)
            for nt in range(NT):
                emit_v(nt)

            # ---- steady head loop ----
            prt = [None] * NT

            def emit_proj1(nt):
                # partial proj over ct 0..3 (heads 0..7) + pbias -> SBUF
                po = psum_big.tile([P, N], F32, tag="pqk", name=f"po1_{nt}")
                for o0, osz in ((0, 512), (512, 256)):
                    for ct in range(CT - 2):
                        nc.tensor.matmul(
                            po[:, o0:o0 + osz],
                            aoT[ct][:, nt * P:(nt + 1) * P],
                            pwT[ct][:, o0:o0 + osz],
                            start=(ct == 0), stop=(ct == CT - 3))
                prt[nt] = prtp.tile([P, DIM], BF16, tag="prt", name=f"prt{nt}")
                nc.vector.tensor_tensor(prt[nt][:], po[:, 0:DIM], pbias[:], op=Add)

            fill = [(t, w) for t in range(2, PAIRS) for w in (0, 1)]
            for h in range(HEADS):
                pav = emit_av(h, exps.pop(h))
                emit_norm(h, pav)
                if h + 2 < HEADS:
                    exps[h + 2] = emit_scores_exp(h + 2)
                if fill:
                    emit_qk_half(*fill.pop(0))
                if h >= 8:
                    emit_proj1(2 * (h - 8))
                    emit_proj1(2 * (h - 8) + 1)

            psum_sc_cm.__exit__(None, None, None)

            # ---- proj stage 2: last two c-tiles (heads 8..11) + partial ----
            for nt in range(NT):
                po = psum_big.tile([P, N], F32, tag="pqk", name=f"po{nt}")
                for o0, osz in ((0, 512), (512, 256)):
                    for ct in (CT - 2, CT - 1):
                        nc.tensor.matmul(
                            po[:, o0:o0 + osz],
                            aoT[ct][:, nt * P:(nt + 1) * P],
                            pwT[ct][:, o0:o0 + osz],
                            start=(ct == CT - 2), stop=(ct == CT - 1))
                ot = outp.tile([P, DIM], F32, tag="out", name=f"out{nt}")
                nc.vector.tensor_tensor(ot[:], po[:, 0:DIM], prt[nt][:], op=Add)
                nc.sync.dma_start(out_d[nt * P:(nt + 1) * P, :], ot[:])

    split_waits(nc)
    return nc


def split_waits(nc):
    """Walrus codegen supports one sync wait per instruction; move extra
    Tile-emitted waits onto EventSemaphore instructions inserted just
    before, in the same engine's program order."""
    n_split = 0
    for bb in nc.m.functions[0].blocks:
        insts = bb.instructions
        new_insts = []
        for inst in insts:
            si = inst.sync_info
            if si is not None and si.on_wait and len(si.on_wait) > 1:
                waits = list(si.on_wait)
                for w in waits[:-1]:
                    ev = mybir.InstEventSemaphore(name=f"{inst.name}-ws{n_split}")
                    ev.engine = inst.engine
                    ev.sync_info = mybir.SyncInfo(on_wait=[w], on_update=[])
                    new_insts.append(ev)
                    n_split += 1
                si.on_wait = [waits[-1]]
                inst.sync_info = si
            new_insts.append(inst)
        if len(new_insts) != len(insts):
            insts[:] = new_insts
    return n_split


_NC_CACHE = None


def get_nc():
    global _NC_CACHE
    if _NC_CACHE is None:
        _NC_CACHE = build_nc()
    return _NC_CACHE


def host_prep(inputs):
    qkv_w = np.asarray(inputs["qkv_w"], dtype=np.float32)
    qkv_b = np.asarray(inputs["qkv_b"], dtype=np.float32)
    proj_w = np.asarray(inputs["proj_w"], dtype=np.float32)
    proj_b = np.asarray(inputs["proj_b"], dtype=np.float32)

    bv = qkv_b[2 * DIM:3 * DIM]
    pb_full = proj_b + proj_w @ bv

    return {
        "qkv_wT": np.ascontiguousarray(qkv_w.T).astype(ml_dtypes.bfloat16),
        "qkv_b": np.ascontiguousarray(qkv_b[0:DIM], dtype=np.float32),
        "proj_wT": np.ascontiguousarray(proj_w.T).astype(ml_dtypes.bfloat16),
        "proj_b": np.ascontiguousarray(pb_full, dtype=np.float32),
    }


def run(inputs, **kwargs):
    nc = get_nc()
    x = np.ascontiguousarray(inputs["x"], dtype=np.float32)
    shared = host_prep(inputs)
    in_maps = [{"x": x[i], **shared} for i in range(N_CORES)]
    res = run_bass_kernel_spmd(nc, in_maps, core_ids=list(range(N_CORES)), **kwargs)
    out = np.stack([res.results[i]["out"] for i in range(N_CORES)], axis=0)
    return out, res


def kernel(x, qkv_w, qkv_b, proj_w, proj_b):
    out, _ = run({"x": x, "qkv_w": qkv_w, "qkv_b": qkv_b,
                  "proj_w": proj_w, "proj_b": proj_b})
    return out
